# revision 16
# baseline (speedup 1.0000x reference)
"""Trainium2 Bass kernel for nn_Attention_43542378447097.

GroupNorm -> multi-head causal self-attention -> out-proj, then the
reference's broadcast add:

    out(B,S,C) + residual(B,C,1,C)  ->  (B,C,S,C)   [right-aligned numpy
    broadcasting, so batches MIX]:

    result[i, j, k, l] = A[j, k, l] + xn[i, j, l]

where A[j] = attention output (incl bo) of batch j and xn[i] = groupnorm
output of batch i.  Output is (96, 96, 96, 96) fp32 (~340 MB) -> memory
bound; ~42.5 MB written per core.

Sharding: core c owns batches/rows i in [12c, 12c+12).
  Phase 1 (per local batch): groupnorm + attention -> A_local (12,96,96)
  Phase 2: AllGather A_local over 8 cores -> A_full (96,96,96), ~3.5 MB
  Phase 3 (per local i): result[i] = A_full + (xn_i + bo_eff) broadcast
    over k -- a single [96, 9216] elementwise add with a stride-0
    middle-dim broadcast on in1 (DVE for 7 slabs, GpSimd for 5), then one
    3.54 MB DMA per slab.

Attention layout choices avoid all cross-partition broadcasts:
  qT/kT per head via lhsT=W-slice, rhs=xnT;  v natural via lhsT=xnT.
  scoresT = kT_h.T @ qT_h  ->  exp on ACT -> causal mask multiply (one
  DVE op over all heads) -> softmax denominators via ones-matmul (sums
  over partitions, result replicated across partitions) -> reciprocal ->
  attnT -> oT_h = v_h.T @ attnT_h -> out = sum_h ocatT_h.T @ Wo_h.
1/sqrt(dk) folded into Wq/bq on host; bv folded into bo_eff = bv@Wo+bo
(softmax rows sum to 1); rstd = exp(-0.5*ln(var+eps)) keeps ACT on one
table set (Ln/Exp/Copy).
"""

import sys

sys.path.insert(0, "/opt/trn_rl_repo")

import numpy as np

B_TOTAL = 96
C = 96
S = 96
NH = 8
DK = 96
G = 8
NCORES = 8
BPC = B_TOTAL // NCORES  # 12
EPS = 1e-5
NFREE = S * C  # 9216
HALFN = NFREE // 2  # assembly half-slab width
# assembly half-slabs 0..23 interleaved between VectorE (16) and GpSimd (8):
# GpSimd is ~2x slower per element and also runs the phase-1 causal masks.
_GPSIMD_HALVES = frozenset(range(1, 24, 3))

_PROG = None


def _build_program(skip_collective=False):
    import concourse.bass as bass
    import concourse.tile as tile
    from concourse import bacc, mybir

    f32 = mybir.dt.float32
    AF = mybir.ActivationFunctionType
    ALU = mybir.AluOpType
    AX = mybir.AxisListType

    nc = bacc.Bacc(
        "TRN2",
        target_bir_lowering=False,
        debug=False,
        enable_asserts=False,
        num_devices=NCORES,
    )

    x_d = nc.declare_dram_parameter("x", [BPC, C, C], f32, isOutput=False)
    # wq/wk carry the bias as a 97th contraction row (paired with a ones row
    # appended to xnT), so q/k evictions are plain copies.
    wq_d = nc.declare_dram_parameter("wq", [C + 1, NH, DK], f32, isOutput=False)
    wk_d = nc.declare_dram_parameter("wk", [C + 1, NH, DK], f32, isOutput=False)
    wv_d = nc.declare_dram_parameter("wv", [C, NH, DK], f32, isOutput=False)
    wo_d = nc.declare_dram_parameter("wo", [DK, NH, C], f32, isOutput=False)
    gamma_d = nc.declare_dram_parameter("gamma", [C, 1], f32, isOutput=False)
    beta_d = nc.declare_dram_parameter("beta", [C, 1], f32, isOutput=False)
    gmask_d = nc.declare_dram_parameter("gmask", [C, C], f32, isOutput=False)
    ones_d = nc.declare_dram_parameter("ones96", [S, S], f32, isOutput=False)
    maskt_d = nc.declare_dram_parameter("maskT", [S, S], f32, isOutput=False)
    iden_d = nc.declare_dram_parameter("iden", [C, C], f32, isOutput=False)
    boe_d = nc.declare_dram_parameter("bo_eff", [1, C], f32, isOutput=False)
    out_d = nc.declare_dram_parameter("out", [BPC, C, NFREE], f32, isOutput=True)

    with tile.TileContext(nc) as tc:
        with (
            tc.tile_pool(name="const", bufs=1) as cpool,
            tc.tile_pool(name="work", bufs=2) as work,
            tc.tile_pool(name="psum", bufs=8, space="PSUM") as pp,
            tc.tile_pool(name="dram", bufs=1, space="DRAM") as dpool,
        ):
            # ---- constants ----
            wq_sb = cpool.tile([C + 1, NH, DK], f32, name="wq_sb")
            wk_sb = cpool.tile([C + 1, NH, DK], f32, name="wk_sb")
            wv_sb = cpool.tile([C, NH, DK], f32, name="wv_sb")
            wo_sb = cpool.tile([DK, NH, C], f32, name="wo_sb")
            gamma_sb = cpool.tile([C, 1], f32, name="gamma_sb")
            beta_sb = cpool.tile([C, 1], f32, name="beta_sb")
            gmask_sb = cpool.tile([C, C], f32, name="gmask_sb")
            ones_sb = cpool.tile([S, S], f32, name="ones_sb")
            maskt_sb = cpool.tile([S, S], f32, name="maskt_sb")
            iden_sb = cpool.tile([C, C], f32, name="iden_sb")
            eps_sb = cpool.tile([C, 1], f32, name="eps_sb")
            bo_rep = cpool.tile([C, C], f32, name="bo_rep")
            xnp_all = cpool.tile([C, BPC, C], f32, name="xnp_all")
            a_sb = cpool.tile([C, NFREE], f32, name="a_sb")

            nc.sync.dma_start(out=wq_sb, in_=wq_d[:])
            nc.sync.dma_start(out=wk_sb, in_=wk_d[:])
            nc.sync.dma_start(out=wv_sb, in_=wv_d[:])
            nc.sync.dma_start(out=wo_sb, in_=wo_d[:])
            nc.sync.dma_start(out=gamma_sb, in_=gamma_d[:])
            nc.sync.dma_start(out=beta_sb, in_=beta_d[:])
            nc.sync.dma_start(out=gmask_sb, in_=gmask_d[:])
            nc.sync.dma_start(out=ones_sb, in_=ones_d[:])
            nc.sync.dma_start(out=maskt_sb, in_=maskt_d[:])
            nc.sync.dma_start(out=iden_sb, in_=iden_d[:])
            nc.sync.dma_start(out=bo_rep, in_=boe_d[:].to_broadcast((C, C)))
            nc.vector.memset(eps_sb, EPS)

            # DRAM bounce buffers for the collective
            a_loc = dpool.tile([BPC, S, C], f32, name="a_loc")
            a_full = dpool.tile(
                [NCORES * BPC, S, C],
                f32,
                name="a_full",
                addr_space="Local" if skip_collective else "Shared",
            )

            inv_n = 1.0 / (C * C // G)  # 1/1152

            # ================= phase 1: local groupnorm + attention ========
            for b in range(BPC):
                x_sb = work.tile([C, C], f32, tag="x_sb")
                nc.sync.dma_start(out=x_sb, in_=x_d[b])

                x2_sb = work.tile([C, C], f32, tag="x2_sb")
                nc.vector.tensor_mul(x2_sb, x_sb, x_sb)
                ps1 = pp.tile([C, C], f32, tag="ps", name="ps_s1")
                nc.tensor.matmul(ps1, lhsT=gmask_sb, rhs=x_sb, start=True, stop=True)
                ps2 = pp.tile([C, C], f32, tag="ps", name="ps_s2")
                nc.tensor.matmul(ps2, lhsT=gmask_sb, rhs=x2_sb, start=True, stop=True)

                s1r = work.tile([C, 1], f32, tag="st", bufs=8, name="s1r")
                s2r = work.tile([C, 1], f32, tag="st", bufs=8, name="s2r")
                nc.vector.tensor_reduce(out=s1r, in_=ps1, axis=AX.X, op=ALU.add)
                nc.vector.tensor_reduce(out=s2r, in_=ps2, axis=AX.X, op=ALU.add)
                mu = work.tile([C, 1], f32, tag="st", bufs=8, name="mu")
                ex2 = work.tile([C, 1], f32, tag="st", bufs=8, name="ex2")
                nc.vector.tensor_scalar_mul(mu, s1r, inv_n)
                nc.vector.tensor_scalar_mul(ex2, s2r, inv_n)
                musq = work.tile([C, 1], f32, tag="st", bufs=8, name="musq")
                nc.vector.tensor_mul(musq, mu, mu)
                var = work.tile([C, 1], f32, tag="st", bufs=8, name="var")
                nc.vector.tensor_sub(var, ex2, musq)
                # rstd = exp(-0.5 * ln(var + eps)); Ln/Exp share one ACT table set
                lnv = work.tile([C, 1], f32, tag="st", bufs=8, name="lnv")
                nc.scalar.activation(
                    out=lnv, in_=var, func=AF.Ln, bias=eps_sb, scale=1.0
                )
                rstd = work.tile([C, 1], f32, tag="st", bufs=8, name="rstd")
                nc.scalar.activation(out=rstd, in_=lnv, func=AF.Exp, scale=-0.5)
                scale_t = work.tile([C, 1], f32, tag="st", bufs=8, name="scale_t")
                nc.vector.tensor_mul(scale_t, rstd, gamma_sb)
                mus = work.tile([C, 1], f32, tag="st", bufs=8, name="mus")
                nc.vector.tensor_mul(mus, mu, scale_t)
                shift_t = work.tile([C, 1], f32, tag="st", bufs=8, name="shift_t")
                nc.vector.tensor_sub(shift_t, beta_sb, mus)

                xn_sb = work.tile([C, C], f32, tag="xn_sb")
                nc.vector.tensor_scalar(
                    xn_sb, x_sb, scale_t, shift_t, op0=ALU.mult, op1=ALU.add
                )
                # residual (+ bo_eff) for the assembly phase
                nc.vector.tensor_add(xnp_all[:, b, :], xn_sb, bo_rep)

                # ---- xnT + ones row (shared rhs for q/k, lhsT for v) ----
                ps_xt = pp.tile([C, C], f32, tag="ps", name="ps_xt")
                nc.tensor.transpose(ps_xt, xn_sb, iden_sb)
                xnT = work.tile([C + 1, C], f32, tag="xnT", name="xnT")
                nc.any.tensor_copy(out=xnT[0:C, :], in_=ps_xt)
                nc.vector.memset(xnT[C : C + 1, :], 1.0)

                # ---- q/k (transposed, bias via 97th row) and v (natural) ----
                qT_sb = work.tile([DK, NH, S], f32, tag="qT_sb")
                kT_sb = work.tile([DK, NH, S], f32, tag="kT_sb")
                v_sb = work.tile([S, NH, DK], f32, tag="v_sb")
                for h in range(NH):
                    psq = pp.tile([DK, S], f32, tag="ps", name="ps_q")
                    nc.tensor.matmul(
                        psq, lhsT=wq_sb[:, h, :], rhs=xnT, start=True, stop=True
                    )
                    nc.any.tensor_copy(out=qT_sb[:, h, :], in_=psq)
                    psk = pp.tile([DK, S], f32, tag="ps", name="ps_k")
                    nc.tensor.matmul(
                        psk, lhsT=wk_sb[:, h, :], rhs=xnT, start=True, stop=True
                    )
                    nc.any.tensor_copy(out=kT_sb[:, h, :], in_=psk)
                    psv = pp.tile([S, DK], f32, tag="ps", name="ps_v")
                    nc.tensor.matmul(
                        psv, lhsT=xnT[0:C, :], rhs=wv_sb[:, h, :], start=True, stop=True
                    )
                    nc.any.tensor_copy(out=v_sb[:, h, :], in_=psv)

                # ---- scoresT -> exp -> causal mask (per head) ----
                expT_sb = work.tile([S, NH, S], f32, tag="expT_sb")
                for h in range(NH):
                    pst = pp.tile([S, S], f32, tag="ps", name="ps_sc")
                    nc.tensor.matmul(
                        pst,
                        lhsT=kT_sb[:, h, :],
                        rhs=qT_sb[:, h, :],
                        start=True,
                        stop=True,
                    )
                    nc.scalar.activation(out=expT_sb[:, h, :], in_=pst, func=AF.Exp)
                    nc.vector.tensor_mul(
                        expT_sb[:, h, :], expT_sb[:, h, :], maskt_sb
                    )

                # ---- softmax denominators (replicated over partitions) ----
                recip_sb = work.tile([S, NH * S], f32, tag="recip_sb")
                for hh in range(2):
                    psd = pp.tile([S, 4 * S], f32, tag="ps", name="ps_den")
                    nc.tensor.matmul(
                        psd,
                        lhsT=ones_sb,
                        rhs=expT_sb[:, 4 * hh : 4 * (hh + 1), :].rearrange(
                            "p h s -> p (h s)"
                        ),
                        start=True,
                        stop=True,
                    )
                    nc.vector.reciprocal(
                        out=recip_sb[:, hh * 4 * S : (hh + 1) * 4 * S], in_=psd
                    )
                nc.vector.tensor_mul(
                    expT_sb,
                    expT_sb,
                    recip_sb.rearrange("p (h s) -> p h s", h=NH),
                )

                # ---- attn @ v (transposed), Wo accumulation ----
                ocatT_sb = work.tile([DK, NH, S], f32, tag="ocatT_sb")
                for h in range(NH):
                    pso = pp.tile([DK, S], f32, tag="ps", name="ps_o")
                    nc.tensor.matmul(
                        pso,
                        lhsT=v_sb[:, h, :],
                        rhs=expT_sb[:, h, :],
                        start=True,
                        stop=True,
                    )
                    nc.any.tensor_copy(out=ocatT_sb[:, h, :], in_=pso)

                psw = pp.tile([S, C], f32, tag="ps", name="ps_w")
                for h in range(NH):
                    nc.tensor.matmul(
                        psw,
                        lhsT=ocatT_sb[:, h, :],
                        rhs=wo_sb[:, h, :],
                        start=(h == 0),
                        stop=(h == NH - 1),
                    )
                outp_sb = work.tile([S, C], f32, tag="outp_sb")
                nc.any.tensor_copy(out=outp_sb, in_=psw)
                nc.sync.dma_start(out=a_loc[b], in_=outp_sb)

            # ================= phase 2: all-gather attention outputs =======
            if skip_collective:
                # timeline-sim variant: approximate the collective's DMA cost
                for cc in range(NCORES):
                    nc.sync.dma_start(
                        out=a_full[cc * BPC : (cc + 1) * BPC], in_=a_loc[:]
                    )
            else:
                nc.gpsimd.collective_compute(
                    "AllGather",
                    mybir.AluOpType.bypass,
                    replica_groups=[list(range(NCORES))],
                    ins=[a_loc.opt()],
                    outs=[a_full.opt()],
                )
            nc.sync.dma_start(
                out=a_sb, in_=a_full[:].rearrange("j k l -> j (k l)")
            )
            a_3d = a_sb.rearrange("p (k l) -> p k l", l=C)

            # ================= phase 3: assemble + write output ============
            # half-slabs interleaved between DVE and GpSimd so both engine
            # streams run concurrently against the output DMA.
            KH = S // 2  # 48 k-rows per half-slab
            for i in range(BPC):
                for half in range(2):
                    g = i * 2 + half
                    res_t = work.tile([C, HALFN], f32, tag="res", bufs=4)
                    eng = nc.gpsimd if g in _GPSIMD_HALVES else nc.vector
                    eng.tensor_tensor(
                        res_t.rearrange("p (k l) -> p k l", l=C),
                        a_3d[:, half * KH : (half + 1) * KH, :],
                        xnp_all[:, i, :].unsqueeze(1).to_broadcast((C, KH, C)),
                        mybir.AluOpType.add,
                    )
                    nc.sync.dma_start(
                        out=out_d[i][:, half * HALFN : (half + 1) * HALFN],
                        in_=res_t,
                    )

    nc.compile()
    return nc


def _get_program():
    global _PROG
    if _PROG is None:
        _PROG = _build_program()
    return _PROG


def _host_inputs(x, Wq, bq, Wk, bk, Wv, bv, Wo, bo, gamma, beta):
    f32 = np.float32
    x = np.asarray(x, f32)
    Wq = np.asarray(Wq, f32)
    bq = np.asarray(bq, f32)
    Wk = np.asarray(Wk, f32)
    bk = np.asarray(bk, f32)
    Wv = np.asarray(Wv, f32)
    bv = np.asarray(bv, f32)
    Wo = np.asarray(Wo, f32)
    bo = np.asarray(bo, f32)
    gamma = np.asarray(gamma, f32)
    beta = np.asarray(beta, f32)

    sc = f32(1.0 / np.sqrt(DK))
    wq97 = np.concatenate(
        [(Wq * sc).reshape(C, NH, DK), (bq * sc).reshape(1, NH, DK)], axis=0
    )
    wk97 = np.concatenate(
        [Wk.reshape(C, NH, DK), bk.reshape(1, NH, DK)], axis=0
    )
    com = {
        "wq": np.ascontiguousarray(wq97),
        "wk": np.ascontiguousarray(wk97),
        "wv": np.ascontiguousarray(Wv.reshape(C, NH, DK)),
        "wo": np.ascontiguousarray(Wo.reshape(NH, DK, C).transpose(1, 0, 2)),
        "gamma": np.ascontiguousarray(gamma.reshape(C, 1)),
        "beta": np.ascontiguousarray(beta.reshape(C, 1)),
        "gmask": np.kron(np.eye(G, dtype=f32), np.ones((C // G, C // G), f32)),
        "ones96": np.ones((S, S), f32),
        "maskT": np.triu(np.ones((S, S), f32)),
        "iden": np.eye(C, dtype=f32),
        "bo_eff": (bv.astype(np.float64) @ Wo.astype(np.float64) + bo)
        .astype(f32)
        .reshape(1, C),
    }
    x_r = np.ascontiguousarray(x.reshape(B_TOTAL, C, C))
    in_maps = []
    for i in range(NCORES):
        m = dict(com)
        m["x"] = np.ascontiguousarray(x_r[i * BPC : (i + 1) * BPC])
        in_maps.append(m)
    return in_maps


def _run(inputs, trace=False):
    from concourse.bass_utils import run_bass_kernel_spmd

    nc = _get_program()
    in_maps = _host_inputs(**inputs)
    res = run_bass_kernel_spmd(
        nc, in_maps, core_ids=list(range(NCORES)), trace=trace
    )
    out = np.concatenate([r["out"] for r in res.results], axis=0)
    return out.reshape(B_TOTAL, C, S, C).astype(np.float32), res


def kernel(**inputs) -> np.ndarray:
    out, _ = _run(inputs, trace=False)
    return out


# revision 19
# speedup vs baseline: 2.0356x; 2.0356x over previous
"""Trainium2 Bass kernel for nn_Attention_43542378447097.

GroupNorm -> multi-head causal self-attention -> out-proj, then the
reference's broadcast add:

    out(B,S,C) + residual(B,C,1,C)  ->  (B,C,S,C)   [right-aligned numpy
    broadcasting, so batches MIX]:

    result[i, j, k, l] = A[j, k, l] + xn[i, j, l]

where A[j] = attention output (incl bo) of batch j and xn[i] = groupnorm
output of batch i.  Output is (96, 96, 96, 96) fp32 (~340 MB) -> memory
bound; ~42.5 MB written per core.

Sharding: core c owns batches/rows i in [12c, 12c+12).
  Phase 1 (per local batch): groupnorm + attention -> A_local (12,96,96)
  Phase 2: AllGather A_local over 8 cores -> A_full (96,96,96), ~3.5 MB
  Phase 3 (per local i): result[i] = A_full + (xn_i + bo_eff) broadcast
    over k -- a single [96, 9216] elementwise add with a stride-0
    middle-dim broadcast on in1 (DVE for 7 slabs, GpSimd for 5), then one
    3.54 MB DMA per slab.

Attention layout choices avoid all cross-partition broadcasts:
  qT/kT per head via lhsT=W-slice, rhs=xnT;  v natural via lhsT=xnT.
  scoresT = kT_h.T @ qT_h  ->  exp on ACT -> causal mask multiply (one
  DVE op over all heads) -> softmax denominators via ones-matmul (sums
  over partitions, result replicated across partitions) -> reciprocal ->
  attnT -> oT_h = v_h.T @ attnT_h -> out = sum_h ocatT_h.T @ Wo_h.
1/sqrt(dk) folded into Wq/bq on host; bv folded into bo_eff = bv@Wo+bo
(softmax rows sum to 1); rstd = exp(-0.5*ln(var+eps)) keeps ACT on one
table set (Ln/Exp/Copy).
"""

import sys

sys.path.insert(0, "/opt/trn_rl_repo")

import numpy as np

B_TOTAL = 96
C = 96
S = 96
NH = 8
DK = 96
G = 8
NCORES = 8
BPC = B_TOTAL // NCORES  # 12
EPS = 1e-5
NFREE = S * C  # 9216
HALFN = NFREE // 2  # assembly half-slab width
# assembly half-slabs 0..23 interleaved between VectorE (16) and GpSimd (8):
# GpSimd is ~2x slower per element and also runs the phase-1 causal masks.
_GPSIMD_HALVES = frozenset(range(1, 24, 3))

_PROG = None


def _build_program(skip_collective=False, loop_n=1):
    import contextlib

    import concourse.bass as bass
    import concourse.tile as tile
    from concourse import bacc, mybir

    f32 = mybir.dt.float32
    AF = mybir.ActivationFunctionType
    ALU = mybir.AluOpType
    AX = mybir.AxisListType

    nc = bacc.Bacc(
        "TRN2",
        target_bir_lowering=False,
        debug=False,
        enable_asserts=False,
        num_devices=NCORES,
    )

    x_d = nc.declare_dram_parameter("x", [BPC, C, C], f32, isOutput=False)
    # wq/wk carry the bias as a 97th contraction row (paired with a ones row
    # appended to xnT), so q/k evictions are plain copies.
    wq_d = nc.declare_dram_parameter("wq", [C + 1, NH, DK], f32, isOutput=False)
    wk_d = nc.declare_dram_parameter("wk", [C + 1, NH, DK], f32, isOutput=False)
    wv_d = nc.declare_dram_parameter("wv", [C, NH, DK], f32, isOutput=False)
    wo_d = nc.declare_dram_parameter("wo", [DK, NH, C], f32, isOutput=False)
    gamma_d = nc.declare_dram_parameter("gamma", [C, 1], f32, isOutput=False)
    beta_d = nc.declare_dram_parameter("beta", [C, 1], f32, isOutput=False)
    gmask_d = nc.declare_dram_parameter("gmask", [C, C], f32, isOutput=False)
    ones_d = nc.declare_dram_parameter("ones96", [S, S], f32, isOutput=False)
    maskt_d = nc.declare_dram_parameter("maskT", [S, S], f32, isOutput=False)
    iden_d = nc.declare_dram_parameter("iden", [C, C], f32, isOutput=False)
    boe_d = nc.declare_dram_parameter("bo_eff", [1, C], f32, isOutput=False)
    out_d = nc.declare_dram_parameter("out", [BPC, C, NFREE], f32, isOutput=True)

    with tile.TileContext(nc) as tc:
        with (
            tc.tile_pool(name="const", bufs=1) as cpool,
            tc.tile_pool(name="work", bufs=2) as work,
            tc.tile_pool(name="psum", bufs=8, space="PSUM") as pp,
            tc.tile_pool(name="dram", bufs=1, space="DRAM") as dpool,
        ):
            # ---- constants ----
            wq_sb = cpool.tile([C + 1, NH, DK], f32, name="wq_sb")
            wk_sb = cpool.tile([C + 1, NH, DK], f32, name="wk_sb")
            wv_sb = cpool.tile([C, NH, DK], f32, name="wv_sb")
            wo_sb = cpool.tile([DK, NH, C], f32, name="wo_sb")
            gamma_sb = cpool.tile([C, 1], f32, name="gamma_sb")
            beta_sb = cpool.tile([C, 1], f32, name="beta_sb")
            gmask_sb = cpool.tile([C, C], f32, name="gmask_sb")
            ones_sb = cpool.tile([S, S], f32, name="ones_sb")
            maskt_sb = cpool.tile([S, S], f32, name="maskt_sb")
            iden_sb = cpool.tile([C, C], f32, name="iden_sb")
            eps_sb = cpool.tile([C, 1], f32, name="eps_sb")
            bo_rep = cpool.tile([C, C], f32, name="bo_rep")
            xnp_all = cpool.tile([C, BPC, C], f32, name="xnp_all")
            a_sb = cpool.tile([C, NFREE], f32, name="a_sb")

            nc.sync.dma_start(out=wq_sb, in_=wq_d[:])
            nc.sync.dma_start(out=wk_sb, in_=wk_d[:])
            nc.sync.dma_start(out=wv_sb, in_=wv_d[:])
            nc.sync.dma_start(out=wo_sb, in_=wo_d[:])
            nc.sync.dma_start(out=gamma_sb, in_=gamma_d[:])
            nc.sync.dma_start(out=beta_sb, in_=beta_d[:])
            nc.sync.dma_start(out=gmask_sb, in_=gmask_d[:])
            nc.sync.dma_start(out=ones_sb, in_=ones_d[:])
            nc.sync.dma_start(out=maskt_sb, in_=maskt_d[:])
            nc.sync.dma_start(out=iden_sb, in_=iden_d[:])
            nc.sync.dma_start(out=bo_rep, in_=boe_d[:].to_broadcast((C, C)))
            nc.vector.memset(eps_sb, EPS)

            # DRAM bounce buffers for the collective
            a_loc = dpool.tile([BPC, S, C], f32, name="a_loc")
            a_full = dpool.tile(
                [NCORES * BPC, S, C],
                f32,
                name="a_full",
                addr_space="Local" if skip_collective else "Shared",
            )

            inv_n = 1.0 / (C * C // G)  # 1/1152

            loop_cm = (
                tc.For_i(0, loop_n, 1)
                if loop_n > 1
                else contextlib.nullcontext()
            )
            loop_cm.__enter__()

            # ================= phase 1: local groupnorm + attention ========
            for b in range(BPC):
                x_sb = work.tile([C, C], f32, tag="x_sb")
                nc.sync.dma_start(out=x_sb, in_=x_d[b])

                x2_sb = work.tile([C, C], f32, tag="x2_sb")
                nc.vector.tensor_mul(x2_sb, x_sb, x_sb)
                ps1 = pp.tile([C, C], f32, tag="ps", name="ps_s1")
                nc.tensor.matmul(ps1, lhsT=gmask_sb, rhs=x_sb, start=True, stop=True)
                ps2 = pp.tile([C, C], f32, tag="ps", name="ps_s2")
                nc.tensor.matmul(ps2, lhsT=gmask_sb, rhs=x2_sb, start=True, stop=True)

                s1r = work.tile([C, 1], f32, tag="st", bufs=8, name="s1r")
                s2r = work.tile([C, 1], f32, tag="st", bufs=8, name="s2r")
                nc.vector.tensor_reduce(out=s1r, in_=ps1, axis=AX.X, op=ALU.add)
                nc.vector.tensor_reduce(out=s2r, in_=ps2, axis=AX.X, op=ALU.add)
                mu = work.tile([C, 1], f32, tag="st", bufs=8, name="mu")
                ex2 = work.tile([C, 1], f32, tag="st", bufs=8, name="ex2")
                nc.vector.tensor_scalar_mul(mu, s1r, inv_n)
                nc.vector.tensor_scalar_mul(ex2, s2r, inv_n)
                musq = work.tile([C, 1], f32, tag="st", bufs=8, name="musq")
                nc.vector.tensor_mul(musq, mu, mu)
                var = work.tile([C, 1], f32, tag="st", bufs=8, name="var")
                nc.vector.tensor_sub(var, ex2, musq)
                # rstd = exp(-0.5 * ln(var + eps)); Ln/Exp share one ACT table set
                lnv = work.tile([C, 1], f32, tag="st", bufs=8, name="lnv")
                nc.scalar.activation(
                    out=lnv, in_=var, func=AF.Ln, bias=eps_sb, scale=1.0
                )
                rstd = work.tile([C, 1], f32, tag="st", bufs=8, name="rstd")
                nc.scalar.activation(out=rstd, in_=lnv, func=AF.Exp, scale=-0.5)
                scale_t = work.tile([C, 1], f32, tag="st", bufs=8, name="scale_t")
                nc.vector.tensor_mul(scale_t, rstd, gamma_sb)
                mus = work.tile([C, 1], f32, tag="st", bufs=8, name="mus")
                nc.vector.tensor_mul(mus, mu, scale_t)
                shift_t = work.tile([C, 1], f32, tag="st", bufs=8, name="shift_t")
                nc.vector.tensor_sub(shift_t, beta_sb, mus)

                xn_sb = work.tile([C, C], f32, tag="xn_sb")
                nc.vector.tensor_scalar(
                    xn_sb, x_sb, scale_t, shift_t, op0=ALU.mult, op1=ALU.add
                )
                # residual (+ bo_eff) for the assembly phase
                nc.vector.tensor_add(xnp_all[:, b, :], xn_sb, bo_rep)

                # ---- xnT + ones row (shared rhs for q/k, lhsT for v) ----
                ps_xt = pp.tile([C, C], f32, tag="ps", name="ps_xt")
                nc.tensor.transpose(ps_xt, xn_sb, iden_sb)
                xnT = work.tile([C + 1, C], f32, tag="xnT", name="xnT")
                nc.any.tensor_copy(out=xnT[0:C, :], in_=ps_xt)
                nc.vector.memset(xnT[C : C + 1, :], 1.0)

                # ---- q/k (transposed, bias via 97th row) and v (natural) ----
                qT_sb = work.tile([DK, NH, S], f32, tag="qT_sb")
                kT_sb = work.tile([DK, NH, S], f32, tag="kT_sb")
                v_sb = work.tile([S, NH, DK], f32, tag="v_sb")
                for h in range(NH):
                    psq = pp.tile([DK, S], f32, tag="ps", name="ps_q")
                    nc.tensor.matmul(
                        psq, lhsT=wq_sb[:, h, :], rhs=xnT, start=True, stop=True
                    )
                    nc.any.tensor_copy(out=qT_sb[:, h, :], in_=psq)
                    psk = pp.tile([DK, S], f32, tag="ps", name="ps_k")
                    nc.tensor.matmul(
                        psk, lhsT=wk_sb[:, h, :], rhs=xnT, start=True, stop=True
                    )
                    nc.any.tensor_copy(out=kT_sb[:, h, :], in_=psk)
                    psv = pp.tile([S, DK], f32, tag="ps", name="ps_v")
                    nc.tensor.matmul(
                        psv, lhsT=xnT[0:C, :], rhs=wv_sb[:, h, :], start=True, stop=True
                    )
                    nc.any.tensor_copy(out=v_sb[:, h, :], in_=psv)

                # ---- scoresT -> exp -> causal mask (per head) ----
                expT_sb = work.tile([S, NH, S], f32, tag="expT_sb")
                for h in range(NH):
                    pst = pp.tile([S, S], f32, tag="ps", name="ps_sc")
                    nc.tensor.matmul(
                        pst,
                        lhsT=kT_sb[:, h, :],
                        rhs=qT_sb[:, h, :],
                        start=True,
                        stop=True,
                    )
                    nc.scalar.activation(out=expT_sb[:, h, :], in_=pst, func=AF.Exp)
                    nc.vector.tensor_mul(
                        expT_sb[:, h, :], expT_sb[:, h, :], maskt_sb
                    )

                # ---- softmax denominators (replicated over partitions) ----
                recip_sb = work.tile([S, NH * S], f32, tag="recip_sb")
                for hh in range(2):
                    psd = pp.tile([S, 4 * S], f32, tag="ps", name="ps_den")
                    nc.tensor.matmul(
                        psd,
                        lhsT=ones_sb,
                        rhs=expT_sb[:, 4 * hh : 4 * (hh + 1), :].rearrange(
                            "p h s -> p (h s)"
                        ),
                        start=True,
                        stop=True,
                    )
                    nc.vector.reciprocal(
                        out=recip_sb[:, hh * 4 * S : (hh + 1) * 4 * S], in_=psd
                    )
                nc.vector.tensor_mul(
                    expT_sb,
                    expT_sb,
                    recip_sb.rearrange("p (h s) -> p h s", h=NH),
                )

                # ---- attn @ v (transposed), Wo accumulation ----
                ocatT_sb = work.tile([DK, NH, S], f32, tag="ocatT_sb")
                for h in range(NH):
                    pso = pp.tile([DK, S], f32, tag="ps", name="ps_o")
                    nc.tensor.matmul(
                        pso,
                        lhsT=v_sb[:, h, :],
                        rhs=expT_sb[:, h, :],
                        start=True,
                        stop=True,
                    )
                    nc.any.tensor_copy(out=ocatT_sb[:, h, :], in_=pso)

                psw = pp.tile([S, C], f32, tag="ps", name="ps_w")
                for h in range(NH):
                    nc.tensor.matmul(
                        psw,
                        lhsT=ocatT_sb[:, h, :],
                        rhs=wo_sb[:, h, :],
                        start=(h == 0),
                        stop=(h == NH - 1),
                    )
                outp_sb = work.tile([S, C], f32, tag="outp_sb")
                nc.any.tensor_copy(out=outp_sb, in_=psw)
                nc.sync.dma_start(out=a_loc[b], in_=outp_sb)

            # ================= phase 2: all-gather attention outputs =======
            if skip_collective:
                # timeline-sim variant: approximate the collective's DMA cost
                for cc in range(NCORES):
                    nc.sync.dma_start(
                        out=a_full[cc * BPC : (cc + 1) * BPC], in_=a_loc[:]
                    )
            else:
                nc.gpsimd.collective_compute(
                    "AllGather",
                    mybir.AluOpType.bypass,
                    replica_groups=[list(range(NCORES))],
                    ins=[a_loc.opt()],
                    outs=[a_full.opt()],
                )
            nc.sync.dma_start(
                out=a_sb, in_=a_full[:].rearrange("j k l -> j (k l)")
            )
            a_3d = a_sb.rearrange("p (k l) -> p k l", l=C)

            # ================= phase 3: assemble + write output ============
            # half-slabs interleaved between DVE and GpSimd so both engine
            # streams run concurrently against the output DMA.
            KH = S // 2  # 48 k-rows per half-slab
            for i in range(BPC):
                for half in range(2):
                    g = i * 2 + half
                    res_t = work.tile([C, HALFN], f32, tag="res", bufs=4)
                    eng = nc.gpsimd if g in _GPSIMD_HALVES else nc.vector
                    eng.tensor_tensor(
                        res_t.rearrange("p (k l) -> p k l", l=C),
                        a_3d[:, half * KH : (half + 1) * KH, :],
                        xnp_all[:, i, :].unsqueeze(1).to_broadcast((C, KH, C)),
                        mybir.AluOpType.add,
                    )
                    nc.sync.dma_start(
                        out=out_d[i][:, half * HALFN : (half + 1) * HALFN],
                        in_=res_t,
                    )

            loop_cm.__exit__(None, None, None)

    nc.compile()
    return nc


def _get_program():
    global _PROG
    if _PROG is None:
        _PROG = _build_program()
    return _PROG


def _host_inputs(x, Wq, bq, Wk, bk, Wv, bv, Wo, bo, gamma, beta):
    f32 = np.float32
    x = np.asarray(x, f32)
    Wq = np.asarray(Wq, f32)
    bq = np.asarray(bq, f32)
    Wk = np.asarray(Wk, f32)
    bk = np.asarray(bk, f32)
    Wv = np.asarray(Wv, f32)
    bv = np.asarray(bv, f32)
    Wo = np.asarray(Wo, f32)
    bo = np.asarray(bo, f32)
    gamma = np.asarray(gamma, f32)
    beta = np.asarray(beta, f32)

    sc = f32(1.0 / np.sqrt(DK))
    wq97 = np.concatenate(
        [(Wq * sc).reshape(C, NH, DK), (bq * sc).reshape(1, NH, DK)], axis=0
    )
    wk97 = np.concatenate(
        [Wk.reshape(C, NH, DK), bk.reshape(1, NH, DK)], axis=0
    )
    com = {
        "wq": np.ascontiguousarray(wq97),
        "wk": np.ascontiguousarray(wk97),
        "wv": np.ascontiguousarray(Wv.reshape(C, NH, DK)),
        "wo": np.ascontiguousarray(Wo.reshape(NH, DK, C).transpose(1, 0, 2)),
        "gamma": np.ascontiguousarray(gamma.reshape(C, 1)),
        "beta": np.ascontiguousarray(beta.reshape(C, 1)),
        "gmask": np.kron(np.eye(G, dtype=f32), np.ones((C // G, C // G), f32)),
        "ones96": np.ones((S, S), f32),
        "maskT": np.triu(np.ones((S, S), f32)),
        "iden": np.eye(C, dtype=f32),
        "bo_eff": (bv.astype(np.float64) @ Wo.astype(np.float64) + bo)
        .astype(f32)
        .reshape(1, C),
    }
    x_r = np.ascontiguousarray(x.reshape(B_TOTAL, C, C))
    in_maps = []
    for i in range(NCORES):
        m = dict(com)
        m["x"] = np.ascontiguousarray(x_r[i * BPC : (i + 1) * BPC])
        in_maps.append(m)
    return in_maps


def _run(inputs, trace=False):
    from concourse.bass_utils import run_bass_kernel_spmd

    nc = _get_program()
    in_maps = _host_inputs(**inputs)
    res = run_bass_kernel_spmd(
        nc, in_maps, core_ids=list(range(NCORES)), trace=trace
    )
    out = np.concatenate([r["out"] for r in res.results], axis=0)
    return out.reshape(B_TOTAL, C, S, C).astype(np.float32), res


def kernel(**inputs) -> np.ndarray:
    out, _ = _run(inputs, trace=False)
    return out
